# revision 1
# baseline (speedup 1.0000x reference)
"""Data-parallel GAT kernel for 8 NeuronCores.

Shards the batch dim of data/loading across the 8 cores (pure data
parallelism per the sharding hint); parameters are replicated. Each
core runs the full 53-node-graph GAT pipeline on its 512-sample shard.
"""
import numpy as np
import jax
import jax.numpy as jnp
from functools import partial

N_NODES = 53
N_HEADS = 8
D_HEAD = 32
NEG_SLOPE_GAT = 0.2
NEG_SLOPE_MLP = 0.01
N_CORES = 8


def _gat_conv(h, fc_w, attn_l, attn_r, bias):
    B = h.shape[0]
    feat = (h @ fc_w.T).reshape(B, N_NODES, N_HEADS, D_HEAD)
    el = jnp.einsum('bnhd,hd->bnh', feat, attn_l)
    er = jnp.einsum('bnhd,hd->bnh', feat, attn_r)
    e = er.transpose(0, 2, 1)[:, :, :, None] + el.transpose(0, 2, 1)[:, :, None, :]
    e = jnp.where(e >= 0, e, NEG_SLOPE_GAT * e)
    # numerically-safe softmax without max-subtraction is fine here
    # (|e| <~ 2 for this data distribution), but keep the stable form
    alpha = jax.nn.softmax(e, axis=-1)
    rst = jnp.einsum('bhqs,bshd->bqhd', alpha, feat)
    rst = rst + h.reshape(B, N_NODES, N_HEADS, D_HEAD)
    rst = rst.reshape(B, N_NODES, N_HEADS * D_HEAD) + bias
    return rst


def _forward(data, loading, linear1_w, linear1_b,
             fc1_w, attn_l1, attn_r1, bias1,
             fc2_w, attn_l2, attn_r2, bias2,
             fc3_w, attn_l3, attn_r3, bias3,
             load_w, load_b, last_w, last_b):
    h = jax.nn.relu(data @ linear1_w.T + linear1_b)
    h = jax.nn.relu(_gat_conv(h, fc1_w, attn_l1, attn_r1, bias1))
    feat1 = jnp.sum(h, axis=1)
    h = jax.nn.relu(_gat_conv(h, fc2_w, attn_l2, attn_r2, bias2))
    feat2 = jnp.sum(h, axis=1)
    h = jax.nn.relu(_gat_conv(h, fc3_w, attn_l3, attn_r3, bias3))
    feat3 = jnp.sum(h, axis=1)
    lf = loading @ load_w.T + load_b
    lf = jnp.where(lf >= 0, lf, NEG_SLOPE_MLP * lf)
    feature = jnp.concatenate([feat1, feat2, feat3, lf], axis=-1)
    return feature @ last_w.T + last_b


_pmapped = None


def _get_pmapped():
    global _pmapped
    if _pmapped is None:
        _pmapped = jax.pmap(
            _forward,
            in_axes=(0, 0) + (None,) * 18,
            devices=jax.devices()[:N_CORES],
        )
    return _pmapped


def kernel(**inputs):
    data = np.asarray(inputs['data'], dtype=np.float32)
    loading = np.asarray(inputs['loading'], dtype=np.float32)
    B = data.shape[0]
    bs = B // N_CORES
    data_sh = data.reshape(N_CORES, bs, N_NODES, data.shape[2])
    loading_sh = loading.reshape(N_CORES, bs, loading.shape[1])

    params = [np.asarray(inputs[k], dtype=np.float32) for k in (
        'linear1_w', 'linear1_b',
        'fc1_w', 'attn_l1', 'attn_r1', 'bias1',
        'fc2_w', 'attn_l2', 'attn_r2', 'bias2',
        'fc3_w', 'attn_l3', 'attn_r3', 'bias3',
        'load_w', 'load_b', 'last_w', 'last_b')]

    fn = _get_pmapped()
    out = fn(data_sh, loading_sh, *params)
    out = np.asarray(jax.device_get(out))
    return out.reshape(B, -1).astype(np.float32)
